# revision 31
# baseline (speedup 1.0000x reference)
"""RWKV-4 WKV attention layer on 8 TRN2 NeuronCores.

Reference computation (T=4096, NE=DA=2048, fp32):
    xx  = shift(x)  (zero-pad first row)
    xk/xv/xr = lerp(xx, x, time_mix_*)
    k, v, r = xk @ Wk, xv @ Wv, xr @ Wr
    wkv = serial scan over T with per-channel decay w = -exp(time_decay),
          bonus u = time_first
    out = (sigmoid(r) * wkv) @ Wo

Distribution strategy (v2 — critical-path optimized):
  - T-shard the projections: core i owns tokens [512i, 512(i+1)); it
    DMA-transposes its x slice to [NE, T] layout, does the time-mix on
    DVE, and computes k/v for ALL channels (activations moving, weights
    stationary).  Channel-blocks are PARITY-grouped into 4 weight strips
    (s0: kt2 0,2,4,6; s1: 8,10,12,14; s2: odds 1..7; s3: 9..15) so the
    AllToAll for channel-half h=0 (even kt2) fires after only 2 strips.
  - AllToAll re-shards k/v by channel; the WKV scan (unstabilized linear
    recurrence via tensor_tensor_scan, numerically safe in fp32/bf16 for
    this input distribution) runs on each core's 256 channels while the
    TENSOR engine computes r.  Scan elementwise work is split across
    DVE / GPSIMD / ACT and runs in place over the kvT tiles.
  - r NEVER crosses cores: sigmoid(r)^T [DA, TSL] stays token-sharded
    (drained from PSUM with a fused Sigmoid), and wkv is exchanged back
    (AllToAll #2) then multiplied by sigmoid(r) locally before the
    output matmul.  This removes one full AllToAll and takes the r
    projection off the scan's critical path.
  - Output matmul accumulates channel blocks evens-first (matching
    exchange arrival order); out slice [512, 2048] fp32 DMA'd out.
  - Host concatenates the 8 output slices.
"""

import math
import os
import sys
from contextlib import ExitStack

for _p in ("/opt/trn_rl_repo", "/root/.axon_site/_ro/trn_rl_repo"):
    if os.path.isdir(_p) and _p not in sys.path:
        sys.path.insert(0, _p)

import numpy as np
import ml_dtypes

import concourse.bass as bass
import concourse.tile as tile
from concourse import bacc, mybir
from concourse.bass_utils import run_bass_kernel_spmd

F32 = mybir.dt.float32
BF16 = mybir.dt.bfloat16
F8 = mybir.dt.float8e4
AL = mybir.AluOpType
ACTF = mybir.ActivationFunctionType
P = 128

# r-projection in fp8 e4m3 with DoubleRow (2x PE throughput).  Host scales
# W_receptance by 2^5 before quantization (values ~N(0, 1/2048) would land
# in the subnormal range otherwise); the sigmoid drain divides it back out.
FP8_R = True
WR_SCALE = 32.0

# channel-block (kt2) permutation: strip s covers KT2[s]; evens first
KT2 = [[0, 2, 4, 6], [8, 10, 12, 14], [1, 3, 5, 7], [9, 11, 13, 15]]
KT_ORDER = [0, 2, 4, 6, 8, 10, 12, 14, 1, 3, 5, 7, 9, 11, 13, 15]


class Cfg:
    def __init__(self, T=4096, NE=2048, DA=2048, NC=8, TH=2048):
        self.T, self.NE, self.DA, self.NC = T, NE, DA, NC
        self.TSL = T // NC          # tokens per core
        self.CSL = DA // NC         # channels per core
        self.NKT = NE // P          # contraction tiles (projections)
        self.NMT = self.TSL // P    # T tiles per slice
        self.NCT = self.CSL // P    # channel ptiles per core
        self.NKT2 = DA // P         # contraction tiles (output matmul)
        self.NOT = NE // 512        # N tiles (output matmul)
        self.TH = min(TH, T)        # scan T-half size
        self.NH = T // self.TH      # number of scan chunks
        assert self.TSL % P == 0 and self.CSL % P == 0
        assert DA % 512 == 0 and NE % 512 == 0 and T % self.TH == 0


def _bcast(ap, n):
    """[P,1] AP -> [P,n] stride-0 broadcast along free."""
    return bass.AP(ap.tensor, ap.offset, [ap.ap[0], [0, n]])


def build_kernel(cfg: Cfg, no_cc: bool = False, reps: int = 1,
                 cc_copy: bool = False, ablate: str | None = None):
    nc = bacc.Bacc("TRN2", target_bir_lowering=False, debug=False,
                   num_devices=1 if no_cc else cfg.NC)

    def _collective(kind, op, replica_groups, ins, outs):
        if no_cc or cc_copy:
            nc.gpsimd.dma_start(out=outs[0], in_=ins[0])
        else:
            nc.gpsimd.collective_compute(kind, op, replica_groups=replica_groups,
                                         ins=ins, outs=outs)
    T, NE, DA, NC = cfg.T, cfg.NE, cfg.DA, cfg.NC
    TSL, CSL = cfg.TSL, cfg.CSL
    RG = [list(range(NC))]

    # x slice staged PRE-TRANSPOSED by the host: [NE, TSL+P] (halo in front)
    xs = nc.declare_dram_parameter("xs", [NE, TSL + P], BF16, isOutput=False)
    wk = nc.declare_dram_parameter("wk", [4 * P, cfg.NKT * 512], BF16, isOutput=False)
    wv = nc.declare_dram_parameter("wv", [4 * P, cfg.NKT * 512], BF16, isOutput=False)
    wr = nc.declare_dram_parameter("wr", [4 * P, cfg.NKT * 512],
                                   F8 if FP8_R else BF16, isOutput=False)
    wo = nc.declare_dram_parameter("wo", [cfg.NOT * P, cfg.NKT2 * 512], BF16, isOutput=False)
    tmk = nc.declare_dram_parameter("tmk", [P, cfg.NKT], F32, isOutput=False)
    tmv = nc.declare_dram_parameter("tmv", [P, cfg.NKT], F32, isOutput=False)
    tmr = nc.declare_dram_parameter("tmr", [P, cfg.NKT], F32, isOutput=False)
    lam = nc.declare_dram_parameter("lam", [P, cfg.NCT], F32, isOutput=False)
    eu = nc.declare_dram_parameter("eu", [P, cfg.NCT], F32, isOutput=False)
    out = nc.declare_dram_parameter("out", [TSL, NE], F32, isOutput=True)

    with tile.TileContext(nc) as tc, ExitStack() as octx:
        dram = octx.enter_context(tc.tile_pool(name="dram", bufs=1, space="DRAM"))
        psum = octx.enter_context(tc.tile_pool(name="psum", bufs=8, space="PSUM"))
        const_pool = octx.enter_context(tc.tile_pool(name="const", bufs=1))
        persist = octx.enter_context(tc.tile_pool(name="persist", bufs=1))
        tokp = octx.enter_context(tc.tile_pool(name="tokp", bufs=2))

        # small constants
        tm_sb = {}
        for name, src in (("k", tmk), ("v", tmv), ("r", tmr)):
            t = const_pool.tile([P, cfg.NKT], F32, tag=f"tm{name}", name=f"tm{name}_sb")
            nc.sync.dma_start(t[:], src[:])
            tm_sb[name] = t
        lam_sb = const_pool.tile([P, cfg.NCT], F32, tag="lam")
        nc.sync.dma_start(lam_sb[:], lam[:])
        eu_sb = const_pool.tile([P, cfg.NCT], F32, tag="eu")
        nc.sync.dma_start(eu_sb[:], eu[:])

        # DRAM bounce buffers for the collectives (shared across reps).
        HDA = NC * P                       # rows per half buffer
        a2a_in = {}
        a2a_out = {}
        for name in ("k", "v"):
            a2a_in[name] = [dram.tile([HDA, TSL], BF16, tag=f"ai_{name}{h}",
                                      name=f"ai_{name}{h}") for h in range(cfg.NCT)]
            a2a_out[name] = [dram.tile([HDA, TSL], BF16, tag=f"ao_{name}{h}",
                                       name=f"ao_{name}{h}") for h in range(cfg.NCT)]
        a2a_in_a = [dram.tile([HDA, TSL], BF16, tag=f"ai_a{h}", name=f"ai_a{h}")
                    for h in range(cfg.NCT)]
        a2a_out_a = [dram.tile([HDA, TSL], BF16, tag=f"ao_a{h}", name=f"ao_a{h}")
                     for h in range(cfg.NCT)]

        # post-A2A channel-sharded tensors [P, T] (bf16), per channel-ptile;
        # the scan runs IN PLACE over these (k -> e^k, v -> num -> wkv)
        kvT = {name: [persist.tile([P, T], BF16, tag=f"{name}T{pt}", name=f"{name}T{pt}")
                      for pt in range(cfg.NCT)] for name in ("k", "v")}
        # sigmoid(r)^T, token-sharded, [128, TSL] per channel block kt2
        srb = persist.tile([P, cfg.NKT2 * TSL], BF16, tag="srb", name="srb")

        prev_osts = None
        for rep in range(reps):
            prev_osts = _emit_body(
                nc, tc, cfg, rep, tm_sb, lam_sb, eu_sb,
                a2a_in, a2a_out, a2a_in_a, a2a_out_a, kvT, srb,
                xs, wk, wv, wr, wo, out, psum, _collective, RG, tokp,
                prev_osts, ablate)

    nc.finalize()
    return nc


def _make_token(nc, tokp, osts, R):
    """Tiny persistent tile whose value depends on all final staging tiles —
    the next rep's gate reads it to serialize bodies for timing."""
    tok = tokp.tile([1, 8], bass.mybir.dt.float32, tag="tok", name=R + "tok")
    for i, o in enumerate(osts):
        nc.vector.tensor_copy(tok[0:1, 2 * (i % 4):2 * (i % 4) + 2],
                              o[0:1, 0:2])
    return tok


def _emit_body(nc, tc, cfg, rep, tm_sb, lam_sb, eu_sb,
               a2a_in, a2a_out, a2a_in_a, a2a_out_a, kvT, srb,
               xs, wk, wv, wr, wo, out, psum, _collective, RG,
               tokp=None, prev_osts=None, ablate=None):
    T, NE, DA, NC = cfg.T, cfg.NE, cfg.DA, cfg.NC
    TSL, CSL, TH, NH = cfg.TSL, cfg.CSL, cfg.TH, cfg.NH
    XW = TSL + P
    R = f"r{rep}_"
    wdram = {"k": wk, "v": wv, "r": wr}
    HKT = cfg.NKT // 2            # kt tiles per weight half-strip
    mixes = {"k": [], "v": [], "r": []}
    slabp_box = [None]

    def load_half(pool, name, s, half, queue):
        if name == "r" and FP8_R:
            wt = pool.tile([P, HKT * 512], F8, tag="wst8",
                           name=R + f"w_{name}_{s}_{half}")
        else:
            wt = pool.tile([P, HKT * 512], BF16, tag="wst",
                           name=R + f"w_{name}_{s}_{half}")
        queue.dma_start(
            wt[:], wdram[name][P * s: P * (s + 1),
                               HKT * 512 * half: HKT * 512 * (half + 1)])
        return wt

    def strip_mms(name, s, wts):
        """matmuls + drain/staging for weight strip s of projection."""
        pts = [psum.tile([P, TSL], F32, tag="pp",
                         name=R + f"ps_{name}_{s}_{c4}")
               for c4 in range(4)]
        if name == "r" and FP8_R:
            # DoubleRow fp8: contraction supertile q covers rows
            # [256q, 256q+256); pair slot s2 = rows [+128s2, +128(s2+1)).
            # lhsT [K=128, 2, M=128], rhs [K=128, 2, N=TSL].
            NQ = cfg.NKT // 2
            HQ = NQ // 2
            for q in range(NQ):
                wt = wts[q // HQ][:, :]
                qo = q % HQ
                r_ = mixes["r"][q][:, :]
                rhs = bass.AP(r_.tensor, r_.offset,
                              [r_.ap[0], [TSL, 2], [1, TSL]])
                for c4 in range(4):
                    lhsT = bass.AP(wt.tensor,
                                   wt.offset + qo * 1024 + c4 * 256,
                                   [wt.ap[0], [128, 2], [1, 128]])
                    nc.tensor.matmul(
                        pts[c4][:], lhsT, rhs,
                        start=(q == 0), stop=(q == NQ - 1),
                        perf_mode=mybir.MatmulPerfMode.DoubleRow)
        else:
            for kt in range(cfg.NKT):
                wt = wts[kt // HKT]
                ko = kt % HKT
                for c4 in range(4):
                    nc.tensor.matmul(
                        pts[c4][:],
                        wt[:, ko * 512 + 128 * c4: ko * 512 + 128 * (c4 + 1)],
                        mixes[name][kt][:, :],
                        start=(kt == 0), stop=(kt == cfg.NKT - 1))
        if name == "r":
            for c4 in range(4):
                kt2 = KT2[s][c4]
                nc.scalar.activation(srb[:, TSL * kt2: TSL * (kt2 + 1)],
                                     pts[c4][:], ACTF.Sigmoid,
                                     scale=(1.0 / WR_SCALE) if FP8_R else 1.0)
        else:
            slab = slabp_box[0].tile([P, 4 * TSL], BF16, tag="slab",
                                     name=R + f"sl_{name}_{s}")
            for c4 in range(4):
                nc.scalar.copy(slab[:, TSL * c4: TSL * (c4 + 1)],
                               pts[c4][:])
            # c4 block -> dest rank j = KT2[s][c4]//2 = 4*(s%2)+c4,
            # ptile-half h = s//2 (strips 0,1 even kt2; 2,3 odd)
            h = 0 if s < 2 else 1
            j0 = 4 * (s % 2)
            dsth = a2a_in[name][h][:]
            dst3 = bass.AP(dsth.tensor, dsth.offset + j0 * P * TSL,
                           [[TSL, P], [P * TSL, 4], [1, TSL]])
            sb = slab[:, :]
            src3 = bass.AP(sb.tensor, sb.offset,
                           [sb.ap[0], [TSL, 4], [1, TSL]])
            nc.sync.dma_start(dst3, src3)

    def exchange(name, h):
        _collective("AllToAll", AL.bypass, replica_groups=RG,
                    ins=[a2a_in[name][h][:].opt()],
                    outs=[a2a_out[name][h][:].opt()])
        s = a2a_out[name][h][:]
        src3 = bass.AP(s.tensor, s.offset,
                       [[TSL, P], [P * TSL, NC], [1, TSL]])
        nc.sync.dma_start(kvT[name][h][:], src3)

    # ------- scan: ACT{exp}, GPSIMD{ekv, num, den, y}, DVE{P,Q,recip} -------
    scan_state = {}

    def scan_unit(phB, pt, h):
        ts_, te = h * TH, (h + 1) * TH
        lam_b = _bcast(lam_sb[:, pt:pt + 1], TH)
        eu_ap = eu_sb[:, pt:pt + 1]
        ek = kvT["k"][pt][:, ts_:te]          # in place: k -> e^k
        nc.scalar.activation(ek, kvT["k"][pt][:, ts_:te], ACTF.Exp)
        ekv = kvT["v"][pt][:, ts_:te]         # in place: v -> e^k * v
        nc.gpsimd.tensor_mul(ekv, ek, kvT["v"][pt][:, ts_:te])

        # per-pt tags: half h+1 must land in a different buffer than half h
        # (the carry copy reads the old tile while writing the new one)
        Pst = phB.tile([P, TH + 1], BF16, tag=f"Pst{pt}")
        Qst = phB.tile([P, TH + 1], BF16, tag=f"Qst{pt}")
        if h == 0:
            nc.gpsimd.memset(Pst[:, 0:1], 0.0)
            nc.gpsimd.memset(Qst[:, 0:1], 0.0)
        else:
            prevP, prevQ = scan_state[pt]
            nc.gpsimd.tensor_copy(Pst[:, 0:1], prevP[:, TH:TH + 1])
            nc.gpsimd.tensor_copy(Qst[:, 0:1], prevQ[:, TH:TH + 1])
        nc.vector.tensor_tensor_scan(
            Pst[:, 1:TH + 1], lam_b, ekv, Pst[:, 0:1],
            op0=AL.mult, op1=AL.add)
        nc.vector.tensor_tensor_scan(
            Qst[:, 1:TH + 1], lam_b, ek, Qst[:, 0:1],
            op0=AL.mult, op1=AL.add)

        # num (bf16, in place over ekv) on DVE; den fp32 for the reciprocal
        # computed on Pool as two tensor_tensor ops (stt is DVE-only on V3)
        den = phB.tile([P, TH], F32, tag="den")
        nc.vector.scalar_tensor_tensor(
            ekv, ekv, eu_ap, Pst[:, 0:TH], op0=AL.mult, op1=AL.add)
        nc.gpsimd.tensor_mul(den[:], ek, _bcast(eu_ap, TH))
        nc.gpsimd.tensor_add(den[:], den[:], Qst[:, 0:TH])
        nc.vector.reciprocal_approx_fast(den[:], den[:])
        # y = num * 1/den, in place over kvT["v"] (becomes wkv)
        nc.gpsimd.tensor_mul(ekv, ekv, den[:])
        scan_state[pt] = (Pst, Qst)

    def exchange_a(pt):
        dst = a2a_in_a[pt][:]
        dst3 = bass.AP(dst.tensor, dst.offset,
                       [[TSL, P], [P * TSL, NC], [1, TSL]])
        nc.sync.dma_start(dst3, kvT["v"][pt][:, :])
        _collective("AllToAll", AL.bypass, replica_groups=RG,
                    ins=[a2a_in_a[pt][:].opt()],
                    outs=[a2a_out_a[pt][:].opt()])

    # =========== emission ===========
    with tc.tile_pool(name=R + "mxp", bufs=1) as mxp, \
         tc.tile_pool(name=R + "wstp", bufs=4) as wstp, \
         tc.tile_pool(name=R + "wrp", bufs=2) as wrp:

        with tc.tile_pool(name=R + "xtp", bufs=1) as xtp, \
             tc.tile_pool(name=R + "mixp", bufs=1) as mixp, \
             tc.tile_pool(name=R + "slabp", bufs=2) as slabp:
            slabp_box[0] = slabp

            # transpose x slice in 4 chunk tiles for early mix start
            NCH = 4
            ktc = cfg.NKT // NCH
            xtrc = [xtp.tile([P, ktc * XW], BF16, tag=f"xtr{c}",
                             name=R + f"xtr{c}") for c in range(NCH)]
            if rep > 0:
                nc.vector.tensor_copy(xtrc[0][0:1, 0:8], prev_osts[0:1, 0:8])
            for kt in range(cfg.NKT):
                c, o = kt // ktc, (kt % ktc) * XW
                nc.sync.dma_start(xtrc[c][:, o: o + XW],
                                  xs[P * kt: P * (kt + 1), :])

            def xparts(kt):
                t = xtrc[kt // ktc]
                o = (kt % ktc) * XW
                return t[:, o + P: o + XW], t[:, o + P - 1: o + XW - 1]

            # time-mix, k-major so k matmuls are not DVE-throttled
            dts = []
            for kt in range(cfg.NKT):
                xm, xx = xparts(kt)
                d = mixp.tile([P, TSL], BF16, tag=f"d{kt}", name=R + f"d{kt}")
                nc.vector.tensor_sub(d[:], xm, xx)
                dts.append(d)
                mt_ = mxp.tile([P, TSL], BF16, tag=f"mxk{kt}", name=R + f"mxk{kt}")
                nc.vector.scalar_tensor_tensor(
                    mt_[:], d[:], tm_sb["k"][:, kt:kt + 1], xx,
                    op0=AL.mult, op1=AL.add)
                mixes["k"].append(mt_)
            for kt in range(cfg.NKT):
                xm, xx = xparts(kt)
                mt_ = mxp.tile([P, TSL], BF16, tag=f"mxv{kt}",
                               name=R + f"mxv{kt}")
                nc.vector.scalar_tensor_tensor(
                    mt_[:], dts[kt][:], tm_sb["v"][:, kt:kt + 1], xx,
                    op0=AL.mult, op1=AL.add)
                mixes["v"].append(mt_)
            if FP8_R:
                # r mixes quantized to fp8 pair tiles: supertile q holds
                # slot s2 = contraction tile kt = 2q+s2 in free half s2
                for kt in range(cfg.NKT):
                    q, s2 = kt // 2, kt % 2
                    xm, xx = xparts(kt)
                    if s2 == 0:
                        t8 = mxp.tile([P, 2 * TSL], F8, tag=f"mxr{q}",
                                      name=R + f"mxr{q}")
                        mixes["r"].append(t8)
                    nc.vector.scalar_tensor_tensor(
                        mixes["r"][q][:, s2 * TSL:(s2 + 1) * TSL],
                        dts[kt][:], tm_sb["r"][:, kt:kt + 1], xx,
                        op0=AL.mult, op1=AL.add)
            else:
                for kt in range(cfg.NKT):
                    xm, xx = xparts(kt)
                    mt_ = mxp.tile([P, TSL], BF16, tag=f"mxr{kt}",
                                   name=R + f"mxr{kt}")
                    nc.vector.scalar_tensor_tensor(
                        mt_[:], dts[kt][:], tm_sb["r"][:, kt:kt + 1], xx,
                        op0=AL.mult, op1=AL.add)
                    mixes["r"].append(mt_)

            # r strip 0 weights: load early from the sync queue
            wt_r0 = [load_half(wrp, "r", 0, hf, nc.sync) for hf in range(2)]

            # k / v projections with per-parity exchanges
            for name in ("k", "v"):
                for s in range(4):
                    wts = [load_half(wstp, name, s, hf, nc.scalar)
                           for hf in range(2)]
                    strip_mms(name, s, wts)
                    if s == 1:
                        exchange(name, 0)
                    elif s == 3:
                        exchange(name, 1)

            # r strips 1, 2 weights: issue before the scan exps hit scalar
            wt_r1 = [load_half(wstp, "r", 1, hf, nc.scalar) for hf in range(2)]
            wt_r2 = [load_half(wstp, "r", 2, hf, nc.scalar) for hf in range(2)]

        with tc.tile_pool(name=R + "phB", bufs=2) as phB:
            wt_r3 = None
            if ablate != "A":
                for h in range(NH):
                    for pt in range(cfg.NCT):
                        scan_unit(phB, pt, h)
                        if h == NH - 1:
                            if wt_r3 is None:
                                # r strip 3 weights: gpsimd queue, emitted
                                # after the first last-half unit's gp ops
                                wt_r3 = [load_half(wrp, "r", 3, hf, nc.gpsimd)
                                         for hf in range(2)]
                            exchange_a(pt)
            else:
                wt_r3 = [load_half(wrp, "r", 3, hf, nc.gpsimd)
                         for hf in range(2)]

            # r projection (token-sharded; sigmoid into srb, no exchange)
            for s, wts_r in ((0, wt_r0), (1, wt_r1), (2, wt_r2), (3, wt_r3)):
                strip_mms("r", s, wts_r)

        if ablate == "A":
            with tc.tile_pool(name=R + "ostl", bufs=1) as ostl:
                osts = [ostl.tile([P, NE], F32, tag=f"ao{mt}",
                                  name=R + f"ablo{mt}")
                        for mt in range(cfg.NMT)]
                for mt in range(cfg.NMT):
                    nc.scalar.copy(osts[mt][:],
                                   srb[:, NE * (mt % 4): NE * (mt % 4 + 1)])
                    nc.sync.dma_start(out[P * mt: P * (mt + 1), :], osts[mt][:])
                tok = _make_token(nc, tokp, osts, R)
            return tok

    # ---------------- phase C: output matmul -----------------------------
    with tc.tile_pool(name=R + "atbp", bufs=1) as atbp, \
         tc.tile_pool(name=R + "wop", bufs=4) as wop, \
         tc.tile_pool(name=R + "ostl", bufs=1) as ostl:
        # gather exchanged wkv into atb: channel block kt2 = 2j + pt
        atb = atbp.tile([P, cfg.NKT2 * TSL], BF16, tag="atb", name=R + "atb")
        for pt in range(cfg.NCT):
            ab = atb[:, :]
            dst3b = bass.AP(ab.tensor, ab.offset + pt * TSL,
                            [ab.ap[0], [cfg.NCT * TSL, NC], [1, TSL]])
            so = a2a_out_a[pt][:]
            src3 = bass.AP(so.tensor, so.offset,
                           [[TSL, P], [P * TSL, NC], [1, TSL]])
            nc.sync.dma_start(dst3b, src3)

        # wo prefetch first, from scalar (not gpsimd — that queue is blocked
        # behind the last collective; scalar is free once r drains finish)
        wots = []
        for nt in range(cfg.NOT):
            wot = wop.tile([P, cfg.NKT2 * 512], BF16, tag="wo",
                           name=R + f"wo_{nt}")
            nc.scalar.dma_start(wot[:], wo[P * nt: P * (nt + 1), :])
            wots.append(wot)

        # multiply wkv^T by sigmoid(r)^T in place, arrival order
        for idx, kt2 in enumerate(KT_ORDER):
            eng = nc.vector if idx % 2 == 0 else nc.gpsimd
            blk = atb[:, kt2 * TSL: (kt2 + 1) * TSL]
            eng.tensor_mul(blk, blk, srb[:, kt2 * TSL: (kt2 + 1) * TSL])

        osts = [ostl.tile([P, NE], F32, tag=f"ost{mt}", name=R + f"ost{mt}")
                for mt in range(cfg.NMT)]
        for ntp in range(cfg.NOT // 2):
            nts = (2 * ntp, 2 * ntp + 1)
            pts = {(mt_, i_): psum.tile([P, 512], F32, tag="pp",
                                        name=R + f"po_{ntp}_{mt_}_{i_}")
                   for mt_ in range(cfg.NMT) for i_ in range(2)}
            for ki, kt in enumerate(KT_ORDER):
                for mt in range(cfg.NMT):
                    lhsT = atb[:, kt * TSL + P * mt: kt * TSL + P * (mt + 1)]
                    for i_ in range(2):
                        nc.tensor.matmul(
                            pts[(mt, i_)][:], lhsT,
                            wots[nts[i_]][:, 512 * kt: 512 * (kt + 1)],
                            start=(ki == 0), stop=(ki == cfg.NKT2 - 1))
            for mt in range(cfg.NMT):
                for i_ in range(2):
                    nt = nts[i_]
                    nc.scalar.copy(osts[mt][:, 512 * nt: 512 * (nt + 1)],
                                   pts[(mt, i_)][:])
        for mt in range(cfg.NMT):
            nc.sync.dma_start(out[P * mt: P * (mt + 1), :], osts[mt][:])
        tok = _make_token(nc, tokp, osts, R)
    return tok


# ------------------------------------------------------------------------
# host side
# ------------------------------------------------------------------------

_CACHE = {}


def _get_nc(cfg: Cfg):
    key = (cfg.T, cfg.NE, cfg.DA, cfg.NC, cfg.TH)
    if key not in _CACHE:
        _CACHE[key] = build_kernel(cfg)
    return _CACHE[key]


def make_in_maps(cfg: Cfg, x, time_first, time_decay, time_mix_k, time_mix_v,
                 time_mix_r, W_key, W_value, W_receptance, W_output):
    T, NE, DA, NC = cfg.T, cfg.NE, cfg.DA, cfg.NC
    TSL, CSL = cfg.TSL, cfg.CSL
    bf = ml_dtypes.bfloat16

    x = np.asarray(x, np.float32)
    xpad = np.zeros((NE, P + T), bf)
    xpad[:, P:] = x.astype(bf).T

    def tile_w_par(w):
        # [NE, DA] -> [4*P, NKT*512] parity strips: strip s, contraction
        # tile kt, block b holds W[128kt+p, 128*KT2[s][b]+c]
        w = np.asarray(w, np.float32).astype(bf)
        w4 = w.reshape(cfg.NKT, P, cfg.NKT2, P)   # [kt, p, kt2, c]
        outw = np.empty((4 * P, cfg.NKT * 512), bf)
        for s in range(4):
            blk = w4[:, :, KT2[s], :]             # [kt, p, 4, c]
            outw[P * s: P * (s + 1)] = (
                blk.transpose(1, 0, 2, 3).reshape(P, cfg.NKT * 512))
        return np.ascontiguousarray(outw)

    def tile_w(w, nkt, ng):
        # [DA, NE] -> [NG*P, NKT*512]: strip g rows hold W[128kt+p, 512g+c]
        w = np.asarray(w, np.float32).astype(bf)
        return np.ascontiguousarray(
            w.reshape(nkt, P, ng, 512).transpose(2, 1, 0, 3)
            .reshape(ng * P, nkt * 512))

    def tile_w_par8(w, scale=WR_SCALE):
        # fp8 DoubleRow layout: strip s row p, element
        # q*1024 + c4*256 + s2*128 + m = W[256q + 128*s2 + p,
        #                                  128*KT2[s][c4] + m] * scale
        f8 = ml_dtypes.float8_e4m3
        w = np.asarray(w, np.float64) * scale
        w4 = np.clip(w, -240, 240).astype(np.float32).reshape(
            cfg.NKT, P, cfg.NKT2, P)
        outw = np.empty((4 * P, cfg.NKT * 512), f8)
        for s in range(4):
            blk = w4[:, :, KT2[s], :]             # [kt, p, c4, m]
            blk = blk.reshape(cfg.NKT // 2, 2, P, 4, P)  # [q, s2, p, c4, m]
            outw[P * s: P * (s + 1)] = (
                blk.transpose(2, 0, 3, 1, 4)
                .reshape(P, cfg.NKT * 512).astype(f8))
        return np.ascontiguousarray(outw)

    wk16 = tile_w_par(W_key)
    wv16 = tile_w_par(W_value)
    wr16 = tile_w_par8(W_receptance) if FP8_R else tile_w_par(W_receptance)
    wo16 = tile_w(W_output, cfg.NKT2, cfg.NOT)

    def col_fold(v, n_t):  # [n_t*P] -> [P, n_t]
        return np.ascontiguousarray(
            np.asarray(v, np.float64).reshape(-1)[: n_t * P]
            .reshape(n_t, P).T.astype(np.float32))

    tmk_a = col_fold(time_mix_k, cfg.NKT)
    tmv_a = col_fold(time_mix_v, cfg.NKT)
    tmr_a = col_fold(time_mix_r, cfg.NKT)

    td = np.asarray(time_decay, np.float64).reshape(-1)
    lam_full = np.exp(-np.exp(td))
    eu_full = np.exp(np.asarray(time_first, np.float64).reshape(-1))

    in_maps = []
    for i in range(NC):
        xsl = np.ascontiguousarray(xpad[:, TSL * i: TSL * i + TSL + P])
        lam_i = np.ascontiguousarray(
            lam_full[CSL * i: CSL * (i + 1)].reshape(cfg.NCT, P).T
            .astype(np.float32))
        eu_i = np.ascontiguousarray(
            eu_full[CSL * i: CSL * (i + 1)].reshape(cfg.NCT, P).T
            .astype(np.float32))
        in_maps.append({
            "xs": xsl, "wk": wk16, "wv": wv16, "wr": wr16, "wo": wo16,
            "tmk": tmk_a, "tmv": tmv_a, "tmr": tmr_a,
            "lam": lam_i, "eu": eu_i,
        })
    return in_maps


def kernel(x, time_first, time_decay, time_mix_k, time_mix_v, time_mix_r,
           W_key, W_value, W_receptance, W_output, _trace=False):
    cfg = Cfg(T=int(np.asarray(x).shape[0]), NE=int(np.asarray(x).shape[1]),
              DA=int(np.asarray(time_decay).reshape(-1).shape[0]), NC=8)
    nc = _get_nc(cfg)
    in_maps = make_in_maps(cfg, x, time_first, time_decay, time_mix_k,
                           time_mix_v, time_mix_r, W_key, W_value,
                           W_receptance, W_output)
    res = run_bass_kernel_spmd(nc, in_maps, core_ids=list(range(cfg.NC)),
                               trace=_trace)
    outp = np.concatenate([res.results[i]["out"] for i in range(cfg.NC)], axis=0)
    out_final = outp.astype(np.float32)
    if _trace:
        return out_final, res
    return out_final
